# revision 20
# baseline (speedup 1.0000x reference)
"""Guided channel-wise 3x3 conv (per-pixel weights) on 8 Trainium2 cores.

out[b,c,h,w] = sum_{dh,dw in {-1,0,1}} input[b,c,h+dh,w+dw] * weights[b,c,k(dh,dw),h,w]
with SAME zero padding.  Shapes: input (8,64,128,128) f32,
weights (8,64,9,128,128) f32 -> out (8,64,128,128) f32.

Sharding: pure data parallelism, one batch sample per NeuronCore (B=8 cores).

All on-device data is fp16 (host casts; output upcast on host): halves HBM
traffic and doubles DVE throughput (2x packed mode needs 16-bit + 4B-aligned
step-1 operands).

Per-core layout: 128 SBUF partitions = (half, c) with p = half*64 + c; each
partition holds one 64-row half of one channel plane, host-padded to 66x130.
Weights/accumulator use a flat 64x130 geometry (zero weight pad in columns
0/129) so every DVE op is a single contiguous 1-D run.  A second on-chip
input copy shifted right by one element (in_s[p,m] = in_pad[p,m-1], built by
the otherwise-idle ACT engine) keeps the odd-offset column taps 4B-aligned:
    dw=1 taps read in_pad[dh*130 : ...]
    dw=0 taps read in_s[dh*130 : ...]
    dw=2 taps read in_s[dh*130+2 : ...]

Synchronization: every DMA dependency group has its OWN semaphore (inc 16 by
its last DMA, wait >= 16*count-of-that-sem's-DMAs).  A cumulative count over
many DMAs on one semaphore is racy: the 16 SDMA engines inc independently,
so a prefix count can be reached while a lagging engine still hasn't
finished an early DMA (observed as NaN on partitions 64..67/96..99).

GpSimd is unused: concurrent Pool+DVE execution degrades DVE ~4x (SBUF port
interference, measured 2.4ns/elem vs 0.52).
"""

import numpy as np

from concourse import bass, mybir
from concourse.bass_utils import run_bass_kernel_spmd

B, CI, H, W = 8, 64, 128, 128
K = 9
HH = H // 2  # rows per half-plane (64)
PR = HH + 2  # padded rows per partition (66)
PC = W + 2  # padded cols (130)
NP = 128  # SBUF partitions
PF = PR * PC  # padded input elems per partition (8580)
FF = HH * PC  # flat padded plane elems per partition (8320)
HF = FF // 2  # 4160

F16 = mybir.dt.float16

# Tap order: dw=1 taps (read in_pad) first so the ACT-built shifted copy has
# time; then dw=0, then dw=2 (read in_s).  (k, buffer, start_offset)
TAPS = [
    (4, "p", 130),  # t0: dh=1, dw=1  (mult-only, initializes out_t)
    (1, "p", 0),  # t1: dh=0, dw=1
    (7, "p", 260),  # t2: dh=2, dw=1
    (3, "s", 130),  # t3: dh=1, dw=0
    (0, "s", 0),  # t4: dh=0, dw=0
    (6, "s", 260),  # t5: dh=2, dw=0
    (5, "s", 132),  # t6: dh=1, dw=2
    (2, "s", 2),  # t7: dh=0, dw=2
    (8, "s", 262),  # t8: dh=2, dw=2
]

SLOT = [0, 1, 2, 3, 4, 0, 1, 2, 3]  # weight slot per tap (5 slots)
# dve_sem ops (1 inc each): interleaved ramp [t0q, t1q-m, t1q-a] x4 = 1..12,
# t2..t7 (m,a) 13..24, t8 quarter-(m,a) 25..32
SLOT_GATE = {5: 10, 6: 12, 7: 14, 8: 16}  # dve value freeing the reused slot
QF = FF // 4  # 2080, quarter plane
IQ = [0, 18 * PC, 34 * PC, 50 * PC, PF]  # input quarter boundaries (rows)


def build_bass():
    nc = bass.Bass()
    inp = nc.declare_dram_parameter("input", [NP, PF], F16, isOutput=False)
    wts = nc.declare_dram_parameter("weights", [K, NP, FF], F16, isOutput=False)
    out = nc.declare_dram_parameter("out", [NP, FF], F16, isOutput=True)

    from contextlib import ExitStack

    with ExitStack() as ctx:
        in_pad = ctx.enter_context(nc.sbuf_tensor("in_pad", [NP, PF], F16))
        in_s = ctx.enter_context(nc.sbuf_tensor("in_s", [NP, PF + 2], F16))
        slots = [
            ctx.enter_context(nc.sbuf_tensor(f"wt{i}", [NP, FF], F16))
            for i in range(5)
        ]
        tmp = ctx.enter_context(nc.sbuf_tensor("tmp", [NP, FF], F16))
        out_t = ctx.enter_context(nc.sbuf_tensor("out_t", [NP, FF], F16))
        block = ctx.enter_context(nc.Block())

        sems = {}
        for name in (
            "q0", "q1", "q2", "q3", "t1q0", "t1q1", "t1q2", "t1q3",
            "t2", "t3", "t4", "t5", "t6", "t7", "t8",
            "cp", "st", "dve",
        ):
            sems[name] = ctx.enter_context(nc.semaphore(f"s_{name}"))
        dve = sems["dve"]

        def src_ap(t, a, b):
            kk, buf, off = TAPS[t]
            return (in_pad if buf == "p" else in_s)[:, off + a : off + b]

        @block.sync
        def _(sync):
            def dma(dst, src, sem):
                sync.dma_start(out=dst, in_=src).then_inc(sems[sem], 16)

            # ramp: triples [input-quarter, t0-wt-quarter, t1-wt-quarter] so
            # DVE has mult+add work while the front-load streams.  inq+t0q
            # share a semaphore (wait >= 32 = both done; ring FIFO also
            # covers every earlier DMA).
            for q in range(4):
                dma(in_pad[:, IQ[q] : IQ[q + 1]], inp[:, IQ[q] : IQ[q + 1]], f"q{q}")
                dma(slots[0][:, q * QF : (q + 1) * QF], wts[TAPS[0][0], :, q * QF : (q + 1) * QF], f"q{q}")
                dma(slots[1][:, q * QF : (q + 1) * QF], wts[TAPS[1][0], :, q * QF : (q + 1) * QF], f"t1q{q}")
            dma(slots[2][:], wts[TAPS[2][0]], "t2")
            dma(slots[3][:], wts[TAPS[3][0]], "t3")
            dma(slots[4][:], wts[TAPS[4][0]], "t4")
            for t in (5, 6, 7, 8):
                sync.wait_ge(dve, SLOT_GATE[t])
                dma(slots[SLOT[t]][:], wts[TAPS[t][0]], f"t{t}")

        @block.scalar
        def _(scalar):
            # build the shifted input copy: in_s[p, m] = in_pad[p, m-1]
            scalar.wait_ge(sems["q3"], 32)  # all input quarters landed
            scalar.activation(
                out=in_s[:, 1 : PF + 1],
                in_=in_pad[:],
                func=mybir.ActivationFunctionType.Copy,
            ).then_inc(sems["cp"], 1)
            # output stores in quarters, streamed as t8's adds complete
            for q in range(4):
                scalar.wait_ge(dve, 26 + 2 * q)
                scalar.dma_start(
                    out=out[:, q * QF : (q + 1) * QF],
                    in_=out_t[:, q * QF : (q + 1) * QF],
                ).then_inc(sems["st"], 16)
            scalar.wait_ge(sems["st"], 64)

        @block.vector
        def _(vector):
            def tt(o, i0, i1, op):
                return vector.tensor_tensor(out=o, in0=i0, in1=i1, op=op)

            MUL, ADD = mybir.AluOpType.mult, mybir.AluOpType.add

            # interleaved ramp: t0 quarter (mult into out_t), then t1 quarter
            # (mult + add) - gives DVE work while the front-load streams
            for q in range(4):
                a, b = q * QF, (q + 1) * QF
                vector.wait_ge(sems[f"q{q}"], 32)
                tt(out_t[:, a:b], src_ap(0, a, b), slots[0][:, a:b], MUL).then_inc(dve, 1)
                vector.wait_ge(sems[f"t1q{q}"], 16)
                tt(tmp[:, a:b], src_ap(1, a, b), slots[1][:, a:b], MUL).then_inc(dve, 1)
                tt(out_t[:, a:b], out_t[:, a:b], tmp[:, a:b], ADD).then_inc(dve, 1)
            # t2: whole
            vector.wait_ge(sems["t2"], 16)
            tt(tmp[:], src_ap(2, 0, FF), slots[2][:], MUL).then_inc(dve, 1)
            tt(out_t[:], out_t[:], tmp[:], ADD).then_inc(dve, 1)
            # t3 also needs the ACT-built shifted copy
            vector.wait_ge(sems["cp"], 1)
            for t in (3, 4, 5, 6, 7):
                vector.wait_ge(sems[f"t{t}"], 16)
                tt(tmp[:], src_ap(t, 0, FF), slots[SLOT[t]][:], MUL).then_inc(dve, 1)
                tt(out_t[:], out_t[:], tmp[:], ADD).then_inc(dve, 1)
            # t8: quarters so the output stores stream behind the adds
            vector.wait_ge(sems["t8"], 16)
            for q in range(4):
                a, b = q * QF, (q + 1) * QF
                tt(tmp[:, a:b], src_ap(8, a, b), slots[SLOT[8]][:, a:b], MUL).then_inc(dve, 1)
                tt(out_t[:, a:b], out_t[:, a:b], tmp[:, a:b], ADD).then_inc(dve, 1)

    return nc


def _prep_input(x):
    """(64,128,128) f32 -> (128, 66*130) fp16 per-partition padded layout."""
    pad = np.zeros((CI, H + 2, W + 2), dtype=np.float16)
    pad[:, 1 : H + 1, 1 : W + 1] = x.astype(np.float16)
    win = np.stack([pad[:, 0:PR, :], pad[:, HH : HH + PR, :]], axis=0)  # (2,64,66,130)
    return np.ascontiguousarray(win.reshape(NP, PF))


def _prep_weights(w):
    """(64,9,128,128) f32 -> (9, 128, 64*130) fp16, zero pad cols 0/129."""
    wp = np.zeros((CI, K, 2, HH, PC), dtype=np.float16)
    wp[:, :, :, :, 1 : W + 1] = w.astype(np.float16).reshape(CI, K, 2, HH, W)
    wr = wp.transpose(1, 2, 0, 3, 4)  # (9, 2, 64, 64, 130)
    return np.ascontiguousarray(wr.reshape(K, NP, FF))


def _unprep_out(o):
    """(128, 64*130) fp16 -> (64,128,128) f32 (strip pad cols)."""
    o = o.astype(np.float32).reshape(2, CI, HH, PC)[:, :, :, 1 : W + 1]
    return np.ascontiguousarray(o.transpose(1, 0, 2, 3).reshape(CI, H, W))


_NC = None


def _get_nc():
    global _NC
    if _NC is None:
        _NC = build_bass()
    return _NC


def make_in_maps(input, weights):
    input = np.asarray(input, dtype=np.float32)
    weights = np.asarray(weights, dtype=np.float32)
    return [
        {"input": _prep_input(input[b]), "weights": _prep_weights(weights[b])}
        for b in range(B)
    ]


def kernel(input, weights):
    nc = _get_nc()
    in_maps = make_in_maps(input, weights)
    res = run_bass_kernel_spmd(nc, in_maps, list(range(B)))
    return np.stack([_unprep_out(res.results[b]["out"]) for b in range(B)], axis=0)


# revision 25
# speedup vs baseline: 1.0486x; 1.0486x over previous
"""Guided channel-wise 3x3 conv (per-pixel weights) on 8 Trainium2 cores.

out[b,c,h,w] = sum_{dh,dw in {-1,0,1}} input[b,c,h+dh,w+dw] * weights[b,c,k(dh,dw),h,w]
with SAME zero padding.  Shapes: input (8,64,128,128) f32,
weights (8,64,9,128,128) f32 -> out (8,64,128,128) f32.

Sharding: pure data parallelism, one batch sample per NeuronCore (B=8 cores).

All on-device data is fp16 (host casts; output upcast on host): halves HBM
traffic and doubles DVE throughput (2x packed mode needs 16-bit + 4B-aligned
step-1 operands).

Per-core layout: 128 SBUF partitions = (half, c) with p = half*64 + c; each
partition holds one 64-row half of one channel plane, host-padded to 66x130.
Weights/accumulator use a flat 64x130 geometry (zero weight pad in columns
0/129) so every DVE op is a single contiguous 1-D run.  A second on-chip
input copy shifted right by one element (in_s[p,m] = in_pad[p,m-1], built by
the otherwise-idle ACT engine) keeps the odd-offset column taps 4B-aligned:
    dw=1 taps read in_pad[dh*130 : ...]
    dw=0 taps read in_s[dh*130 : ...]
    dw=2 taps read in_s[dh*130+2 : ...]

Synchronization: every DMA dependency group has its OWN semaphore (inc 16 by
its last DMA, wait >= 16*count-of-that-sem's-DMAs).  A cumulative count over
many DMAs on one semaphore is racy: the 16 SDMA engines inc independently,
so a prefix count can be reached while a lagging engine still hasn't
finished an early DMA (observed as NaN on partitions 64..67/96..99).

GpSimd is unused: concurrent Pool+DVE execution degrades DVE ~4x (SBUF port
interference, measured 2.4ns/elem vs 0.52).
"""

import numpy as np

from concourse import bass, mybir
from concourse.bass_utils import run_bass_kernel_spmd

B, CI, H, W = 8, 64, 128, 128
K = 9
HH = H // 2  # rows per half-plane (64)
PR = HH + 2  # padded rows per partition (66)
PC = W + 2  # padded cols (130)
NP = 128  # SBUF partitions
PF = PR * PC  # padded input elems per partition (8580)
FF = HH * PC  # flat padded plane elems per partition (8320)
HF = FF // 2  # 4160

F16 = mybir.dt.float16

# Tap order: dw=1 taps (read in_pad) first so the ACT-built shifted copy has
# time; then dw=0, then dw=2 (read in_s).  (k, buffer, start_offset)
TAPS = [
    (4, "p", 130),  # t0: dh=1, dw=1  (mult-only, initializes out_t)
    (1, "p", 0),  # t1: dh=0, dw=1
    (7, "p", 260),  # t2: dh=2, dw=1
    (3, "s", 130),  # t3: dh=1, dw=0
    (0, "s", 0),  # t4: dh=0, dw=0
    (6, "s", 260),  # t5: dh=2, dw=0
    (5, "s", 132),  # t6: dh=1, dw=2
    (2, "s", 2),  # t7: dh=0, dw=2
    (8, "s", 262),  # t8: dh=2, dw=2
]

SLOT = [0, 1, 2, 3, 4, 0, 1, 2, 3]  # weight slot per tap (5 slots)
# dve_sem ops (1 inc each): interleaved ramp [t0q, t1q-m, t1q-a] x4 = 1..12,
# t2 half-(m,a) 13..16, t3..t7 (m,a) 17..26, t8 quarter-(m,a) 27..34
SLOT_GATE = {5: 10, 6: 12, 7: 16, 8: 18}  # dve value freeing the reused slot
QF = FF // 4  # 2080, quarter plane
IQ = [0, 18 * PC, 34 * PC, 50 * PC, PF]  # input quarter boundaries (rows)


def build_bass():
    nc = bass.Bass()
    inp = nc.declare_dram_parameter("input", [NP, PF], F16, isOutput=False)
    wts = nc.declare_dram_parameter("weights", [K, NP, FF], F16, isOutput=False)
    out = nc.declare_dram_parameter("out", [NP, FF], F16, isOutput=True)

    from contextlib import ExitStack

    with ExitStack() as ctx:
        in_pad = ctx.enter_context(nc.sbuf_tensor("in_pad", [NP, PF], F16))
        in_s = ctx.enter_context(nc.sbuf_tensor("in_s", [NP, PF + 2], F16))
        slots = [
            ctx.enter_context(nc.sbuf_tensor(f"wt{i}", [NP, FF], F16))
            for i in range(5)
        ]
        tmp = ctx.enter_context(nc.sbuf_tensor("tmp", [NP, FF], F16))
        out_t = ctx.enter_context(nc.sbuf_tensor("out_t", [NP, FF], F16))
        block = ctx.enter_context(nc.Block())

        sems = {}
        for name in (
            "q0", "q1", "q2", "q3", "t1q0", "t1q1", "t1q2", "t1q3",
            "t2a", "t2b", "t3", "t4", "t5", "t6", "t7", "t8",
            "cp", "st", "dve",
        ):
            sems[name] = ctx.enter_context(nc.semaphore(f"s_{name}"))
        dve = sems["dve"]

        def src_ap(t, a, b):
            kk, buf, off = TAPS[t]
            return (in_pad if buf == "p" else in_s)[:, off + a : off + b]

        @block.sync
        def _(sync):
            def dma(dst, src, sem):
                sync.dma_start(out=dst, in_=src).then_inc(sems[sem], 16)

            # ramp: triples [input-quarter, t0-wt-quarter, t1-wt-quarter] so
            # DVE has mult+add work while the front-load streams.  inq+t0q
            # share a semaphore (wait >= 32 = both done; ring FIFO also
            # covers every earlier DMA).
            for q in range(4):
                dma(in_pad[:, IQ[q] : IQ[q + 1]], inp[:, IQ[q] : IQ[q + 1]], f"q{q}")
                dma(slots[0][:, q * QF : (q + 1) * QF], wts[TAPS[0][0], :, q * QF : (q + 1) * QF], f"q{q}")
                dma(slots[1][:, q * QF : (q + 1) * QF], wts[TAPS[1][0], :, q * QF : (q + 1) * QF], f"t1q{q}")
            dma(slots[2][:, 0:HF], wts[TAPS[2][0], :, 0:HF], "t2a")
            dma(slots[2][:, HF:FF], wts[TAPS[2][0], :, HF:FF], "t2b")
            dma(slots[3][:], wts[TAPS[3][0]], "t3")
            dma(slots[4][:], wts[TAPS[4][0]], "t4")
            for t in (5, 6, 7, 8):
                sync.wait_ge(dve, SLOT_GATE[t])
                dma(slots[SLOT[t]][:], wts[TAPS[t][0]], f"t{t}")

        @block.scalar
        def _(scalar):
            # build the shifted input copy: in_s[p, m] = in_pad[p, m-1]
            scalar.wait_ge(sems["q3"], 32)  # all input quarters landed
            scalar.activation(
                out=in_s[:, 1 : PF + 1],
                in_=in_pad[:],
                func=mybir.ActivationFunctionType.Copy,
            ).then_inc(sems["cp"], 1)
            # output stores in quarters, streamed as t8's adds complete
            for q in range(4):
                scalar.wait_ge(dve, 28 + 2 * q)
                scalar.dma_start(
                    out=out[:, q * QF : (q + 1) * QF],
                    in_=out_t[:, q * QF : (q + 1) * QF],
                ).then_inc(sems["st"], 16)
            scalar.wait_ge(sems["st"], 64)

        @block.vector
        def _(vector):
            def tt(o, i0, i1, op):
                return vector.tensor_tensor(out=o, in0=i0, in1=i1, op=op)

            MUL, ADD = mybir.AluOpType.mult, mybir.AluOpType.add

            # interleaved ramp: t0 quarter (mult into out_t), then t1 quarter
            # (mult + add) - gives DVE work while the front-load streams
            for q in range(4):
                a, b = q * QF, (q + 1) * QF
                vector.wait_ge(sems[f"q{q}"], 32)
                tt(out_t[:, a:b], src_ap(0, a, b), slots[0][:, a:b], MUL).then_inc(dve, 1)
                vector.wait_ge(sems[f"t1q{q}"], 16)
                tt(tmp[:, a:b], src_ap(1, a, b), slots[1][:, a:b], MUL).then_inc(dve, 1)
                tt(out_t[:, a:b], out_t[:, a:b], tmp[:, a:b], ADD).then_inc(dve, 1)
            # t2: halves (keeps the pipeline prefix balanced)
            vector.wait_ge(sems["t2a"], 16)
            tt(tmp[:, 0:HF], src_ap(2, 0, HF), slots[2][:, 0:HF], MUL).then_inc(dve, 1)
            tt(out_t[:, 0:HF], out_t[:, 0:HF], tmp[:, 0:HF], ADD).then_inc(dve, 1)
            vector.wait_ge(sems["t2b"], 16)
            tt(tmp[:, HF:FF], src_ap(2, HF, FF), slots[2][:, HF:FF], MUL).then_inc(dve, 1)
            tt(out_t[:, HF:FF], out_t[:, HF:FF], tmp[:, HF:FF], ADD).then_inc(dve, 1)
            # t3 also needs the ACT-built shifted copy
            vector.wait_ge(sems["cp"], 1)
            for t in (3, 4, 5, 6, 7):
                vector.wait_ge(sems[f"t{t}"], 16)
                tt(tmp[:], src_ap(t, 0, FF), slots[SLOT[t]][:], MUL).then_inc(dve, 1)
                tt(out_t[:], out_t[:], tmp[:], ADD).then_inc(dve, 1)
            # t8: quarters so the output stores stream behind the adds
            vector.wait_ge(sems["t8"], 16)
            for q in range(4):
                a, b = q * QF, (q + 1) * QF
                tt(tmp[:, a:b], src_ap(8, a, b), slots[SLOT[8]][:, a:b], MUL).then_inc(dve, 1)
                tt(out_t[:, a:b], out_t[:, a:b], tmp[:, a:b], ADD).then_inc(dve, 1)

    return nc


def _prep_input(x):
    """(64,128,128) f32 -> (128, 66*130) fp16 per-partition padded layout."""
    pad = np.zeros((CI, H + 2, W + 2), dtype=np.float16)
    pad[:, 1 : H + 1, 1 : W + 1] = x.astype(np.float16)
    win = np.stack([pad[:, 0:PR, :], pad[:, HH : HH + PR, :]], axis=0)  # (2,64,66,130)
    return np.ascontiguousarray(win.reshape(NP, PF))


def _prep_weights(w):
    """(64,9,128,128) f32 -> (9, 128, 64*130) fp16, zero pad cols 0/129."""
    wp = np.zeros((CI, K, 2, HH, PC), dtype=np.float16)
    wp[:, :, :, :, 1 : W + 1] = w.astype(np.float16).reshape(CI, K, 2, HH, W)
    wr = wp.transpose(1, 2, 0, 3, 4)  # (9, 2, 64, 64, 130)
    return np.ascontiguousarray(wr.reshape(K, NP, FF))


def _unprep_out(o):
    """(128, 64*130) fp16 -> (64,128,128) f32 (strip pad cols)."""
    o = o.astype(np.float32).reshape(2, CI, HH, PC)[:, :, :, 1 : W + 1]
    return np.ascontiguousarray(o.transpose(1, 0, 2, 3).reshape(CI, H, W))


_NC = None


def _get_nc():
    global _NC
    if _NC is None:
        _NC = build_bass()
    return _NC


def make_in_maps(input, weights):
    input = np.asarray(input, dtype=np.float32)
    weights = np.asarray(weights, dtype=np.float32)
    return [
        {"input": _prep_input(input[b]), "weights": _prep_weights(weights[b])}
        for b in range(B)
    ]


def kernel(input, weights):
    nc = _get_nc()
    in_maps = make_in_maps(input, weights)
    res = run_bass_kernel_spmd(nc, in_maps, list(range(B)))
    return np.stack([_unprep_out(res.results[b]["out"]) for b in range(B)], axis=0)
